# revision 27
# baseline (speedup 1.0000x reference)
"""Trainium2 Bass kernel for nn_CrossSRA (spatial-reduction cross-attention).

Sharding (8 NeuronCores):
  - Batch-parallel for the attention path: core b owns batch b.
  - The spatial-reduction conv is split by kernel-row dy across the 8 cores;
    partials are combined with two in-kernel ReduceScatters (token-split so
    RS#1 overlaps the second half of the conv).

Key structure:
  - q-projection is folded into the score matmuls: W_pr = [qw_h^T k_h^T]
    precomputed per head-pair, scores = W^T @ qxT (pair-stacked PSUM).
  - Softmax denominators via one block-diagonal ones matmul per pair.
  - attn@v via 6 block-diagonal v-tile matmuls per chunk.
All matmuls bf16 with fp32 PSUM; layernorm/softmax statistics stay fp32.
"""

import numpy as np
import ml_dtypes

import concourse.bass as bass
import concourse.tile as tile
from concourse import bacc, mybir
from concourse.bass_utils import run_bass_kernel_spmd
from concourse.masks import make_identity

# problem shape (hardcoded per spec)
B = 8
N = 4096
C = 768
H = 8
DH = C // H            # 96
IMG = 64               # h = w = 64
SR = 8
KM = 64                # kv tokens after spatial reduction (8x8)
EPS = 1e-5
SCALE = DH ** -0.5

P = 128
CT = C // P            # 6 channel tiles
NCHUNK = 512
NCH = N // NCHUNK      # 8 column chunks
HKM = KM // 2          # 32 tokens per conv half

BF = mybir.dt.bfloat16
F32 = mybir.dt.float32
BF_NP = ml_dtypes.bfloat16

_CACHE: dict = {}

# output channel tile t is covered by heads (hA, hA+1) split at width WA
def _tile_heads(t):
    c0 = P * t
    hA = c0 // DH
    wA = DH * (hA + 1) - c0
    return hA, wA


def _build_program():
    nc = bacc.Bacc("TRN2", target_bir_lowering=False, debug=False, num_devices=8)

    d_in = {}
    def din(name, shape, dt):
        d_in[name] = nc.dram_tensor(name, shape, dt, kind="ExternalInput").ap()
        return d_in[name]

    qxT = din("qxT", [C, N], BF)          # this batch's qx, transposed
    # all batches' kvx tokens with dy=core, layout [c, half, dx, b, i2, jj]
    kvg = din("kvg", [C, 2 * SR * B * HKM], BF)  # [768, 4096]
    cwT = din("cwT", [SR, C, C], BF)      # conv_w[o, c, dy=core, dx] -> [dx, c, o]
    kvb = din("kvb", [IMG, IMG], F32)     # this batch's kv_bias image
    qwh = din("qwh", [DH, H, C], BF)      # q_w rows grouped per head: [d, h, c]
    kwT = din("kwT", [C, C], BF)
    vwT = din("vwT", [C, C], BF)
    pwT = din("pwT", [C, C], BF)          # proj_w.T
    qb2 = din("qb2", [DH, H], BF)
    kb2 = din("kb2", [DH, H], F32)
    vb = din("vb", [C], F32)
    cb = din("cb", [C], F32)
    pb = din("pb", [C], F32)
    lnw = din("lnw", [C], F32)
    lnb = din("lnb", [C], F32)

    out = nc.dram_tensor("out", [N, C], F32, kind="ExternalOutput").ap()

    def bcast(vec_ap, parts):
        return bass.AP(tensor=vec_ap.tensor, offset=0, ap=[[0, parts], [1, C]])

    with tile.TileContext(nc) as tc:
        import contextlib
        stack = contextlib.ExitStack()
        with stack:
            consts = stack.enter_context(tc.tile_pool(name="consts", bufs=1))
            wpool = stack.enter_context(tc.tile_pool(name="weights", bufs=1))
            dram = stack.enter_context(tc.tile_pool(name="dram", bufs=1, space="DRAM"))

            # ---- constants (scalar DMA queue; sync queue is conv-critical) ----
            ident = consts.tile([KM, KM], F32, tag="ident")
            make_identity(nc, ident[:])
            eps_t = consts.tile([KM, 1], F32, tag="eps")
            nc.vector.memset(eps_t[:], EPS)
            # block-diagonal ones: col c sums rows of its own half
            ones_blk = consts.tile([P, P], BF, tag="onesblk")
            nc.vector.memset(ones_blk[:], 0.0)
            nc.vector.memset(ones_blk[0:KM, 0:KM], 1.0)
            nc.vector.memset(ones_blk[KM:P, KM:P], 1.0)

            vb_b = consts.tile([KM, C], F32, tag="vb")
            nc.scalar.dma_start(vb_b[:], bcast(vb, KM))
            cb_b = consts.tile([KM, C], F32, tag="cb")
            nc.scalar.dma_start(cb_b[:], bcast(cb, KM))
            lnw_b = consts.tile([KM, C], F32, tag="lnw")
            nc.scalar.dma_start(lnw_b[:], bcast(lnw, KM))
            lnb_b = consts.tile([KM, C], F32, tag="lnb")
            nc.scalar.dma_start(lnb_b[:], bcast(lnb, KM))
            pb_b = consts.tile([P, C], F32, tag="pb")
            nc.scalar.dma_start(pb_b[:], bcast(pb, P))
            qb_sb = consts.tile([DH, H], BF, tag="qb")
            nc.scalar.dma_start(qb_sb[:], qb2[:])
            kb_sb = consts.tile([DH, H], F32, tag="kb")
            nc.scalar.dma_start(kb_sb[:], kb2[:])

            # attention bias: 4-point average of the bilinear resize (64->8)
            g4 = consts.tile([8, 8, 2, 2], F32, tag="g4")
            for dy in range(2):
                src = bass.AP(tensor=kvb.tensor, offset=(3 + dy) * IMG + 3,
                              ap=[[8 * IMG, 8], [8, 8], [1, 2]])
                nc.scalar.dma_start(g4[:, :, dy, :], src)
            s4 = consts.tile([8, 8], F32, tag="s4")
            nc.vector.reduce_sum(s4[:], g4[:], axis=mybir.AxisListType.XY)
            s4q = consts.tile([8, 8], F32, tag="s4q")
            nc.scalar.mul(s4q[:], s4[:], 0.25)
            attnb = consts.tile([P, 1], F32, tag="attnb")
            nc.scalar.dma_start(attnb[0:KM, :], s4q[:])   # [8p,8f] -> [64p,1f]
            nc.scalar.dma_start(attnb[KM:P, :], s4q[:])   # duplicate for pair stack

            # collective bounce buffers (bf16 partial sums), rows = (b, m)
            partials = dram.tile([B * KM, C], BF, tag="partials",
                                 name="partials")
            kvred = dram.tile([KM, C], BF, tag="kvred", name="kvred")

            # qx resident for the whole attention phase; its DMAs are queued
            # on sync AFTER the conv stream, weights after that
            qxpool = stack.enter_context(tc.tile_pool(name="qx", bufs=1))
            qx_sb = qxpool.tile([P, CT, N], BF, tag="qxall")

            # ================= conv phase (k-split over dy) =================
            with tc.tile_pool(name="convp", bufs=8) as cvpool, \
                 tc.tile_pool(name="cwres", bufs=1) as cwpool, \
                 tc.tile_pool(name="cvpsum", bufs=2, space="PSUM") as cvps, \
                 tc.tile_pool(name="cvout", bufs=2) as cvo:
                # sync-queue FIFO in consumption order: (cw_dx, kvx_h0_dx)
                # interleaved, then kvx_h1_dx, then qx, then weights
                cw_t = []
                kvx_t = {}
                for dx in range(SR):
                    t = cwpool.tile([P, CT, C], BF, tag=f"cw{dx}", name=f"cw{dx}")
                    kt = cvpool.tile([P, CT, 256], BF, tag="kvx",
                                     name=f"kvx0_{dx}")
                    cw_view = bass.AP(tensor=cwT.tensor, offset=dx * C * C,
                                      ap=[[C, P], [P * C, CT], [1, C]])
                    kv_view = bass.AP(tensor=kvg.tensor, offset=dx * 256,
                                      ap=[[4096, P], [P * 4096, CT], [1, 256]])
                    if dx == 0:
                        # per-ct interleaved so the first matmuls unblock early
                        for ct in range(CT):
                            cw_ct = bass.AP(tensor=cwT.tensor,
                                            offset=dx * C * C + ct * P * C,
                                            ap=[[C, P], [1, C]])
                            nc.sync.dma_start(t[:, ct, :], cw_ct)
                            kv_ct = bass.AP(tensor=kvg.tensor,
                                            offset=dx * 256 + ct * P * 4096,
                                            ap=[[4096, P], [1, 256]])
                            nc.sync.dma_start(kt[:, ct, :], kv_ct)
                    else:
                        nc.sync.dma_start(t[:], cw_view)
                        nc.sync.dma_start(kt[:], kv_view)
                    cw_t.append(t)
                    kvx_t[(0, dx)] = kt
                for dx in range(SR):
                    kt = cvpool.tile([P, CT, 256], BF, tag="kvx",
                                     name=f"kvx1_{dx}")
                    view = bass.AP(tensor=kvg.tensor, offset=2048 + dx * 256,
                                   ap=[[4096, P], [P * 4096, CT], [1, 256]])
                    nc.sync.dma_start(kt[:], view)
                    kvx_t[(1, dx)] = kt
                for half in range(2):
                    cps = [cvps.tile([P, C], F32, tag=f"cv{q}", name=f"cv{half}{q}")
                           for q in range(2)]
                    for dx in range(SR):
                        for q in range(2):
                            for ct in range(CT):
                                lhsT = kvx_t[(half, dx)][:, ct,
                                                         q * P:(q + 1) * P]
                                for o0, osz in ((0, 512), (512, 256)):
                                    nc.tensor.matmul(
                                        cps[q][:, o0:o0 + osz], lhsT,
                                        cw_t[dx][:, ct, o0:o0 + osz],
                                        start=(dx == 0 and ct == 0),
                                        stop=(dx == SR - 1 and ct == CT - 1))
                    for q in range(2):
                        pt = cvo.tile([P, C], BF, tag="cvo")
                        nc.vector.tensor_copy(pt[:], cps[q][:])
                        # psum rows (b-4q, i2, jj) -> partials rows b*64+32h+..
                        dst = bass.AP(
                            tensor=partials[:].tensor,
                            offset=(q * 4 * KM + HKM * half) * C,
                            ap=[[KM * C, 4], [C, HKM], [1, C]])
                        nc.gpsimd.dma_start(dst, pt[:])
                nc.gpsimd.collective_compute(
                    "ReduceScatter", mybir.AluOpType.add,
                    replica_groups=[list(range(8))],
                    ins=[partials.opt()], outs=[kvred.opt()])
                # dedent note: single RS after both halves (see loop below)

            # remaining weights (sync FIFO: after conv stream + qx)
            def load_wT(src_ap, tag):
                t = wpool.tile([P, CT, C], BF, tag=tag)
                view = bass.AP(tensor=src_ap.tensor, offset=0,
                               ap=[[C, P], [P * C, CT], [1, C]])
                nc.sync.dma_start(t[:], view)
                return t

            qwh_sb = wpool.tile([DH, H, C], BF, tag="qwh")
            nc.sync.dma_start(qwh_sb[:], qwh[:])
            kw_sb = load_wT(kwT, "kw")
            vw_sb = load_wT(vwT, "vw")
            pw_sb = load_wT(pwT, "pw")
            for ct in range(CT):
                view = bass.AP(tensor=qxT.tensor, offset=ct * P * N,
                               ap=[[N, P], [1, N]])
                nc.sync.dma_start(qx_sb[:, ct, :], view)

            # ================= kv epilogue =================
            kvpool = stack.enter_context(tc.tile_pool(name="kv", bufs=1))
            kvps_ctx = tc.tile_pool(name="kvpsum", bufs=1, space="PSUM")
            kvps = kvps_ctx.__enter__()
            kvbf = kvpool.tile([KM, C], BF, tag="kvbf")
            nc.gpsimd.dma_start(kvbf[:], kvred[:])
            kv_sb = kvpool.tile([KM, C], F32, tag="kv")
            nc.vector.tensor_add(kv_sb[:], kvbf[:], cb_b[:])
            # layernorm over channels
            BD = nc.vector.BN_STATS_DIM
            stats = kvpool.tile([KM, 3, BD], F32, tag="stats")
            kv_g = kv_sb[:].rearrange("p (g d) -> p g d", g=3)
            for g in range(3):
                nc.vector.bn_stats(stats[:, g, :], kv_g[:, g, :])
            mv = kvpool.tile([KM, nc.vector.BN_AGGR_DIM], F32, tag="mv")
            nc.vector.bn_aggr(mv[:], stats[:])
            nc.scalar.activation(mv[:, 1:2], mv[:, 1:2],
                                 mybir.ActivationFunctionType.Sqrt, bias=eps_t[:])
            nc.vector.reciprocal(mv[:, 1:2], mv[:, 1:2])
            nc.vector.tensor_scalar(kv_sb[:], kv_sb[:],
                                    scalar1=mv[:, 0:1], scalar2=mv[:, 1:2],
                                    op0=mybir.AluOpType.subtract,
                                    op1=mybir.AluOpType.mult)
            nc.vector.tensor_mul(kv_sb[:], kv_sb[:], lnw_b[:])
            nc.vector.tensor_add(kv_sb[:], kv_sb[:], lnb_b[:])

            # transpose kv -> kvT [c, m]
            kvT_sb = kvpool.tile([P, CT, KM], BF, tag="kvT")
            for ct in range(CT):
                ptr = kvps.tile([P, KM], F32, tag="ptr")
                nc.tensor.transpose(ptr[:], kv_sb[:, ct * P:(ct + 1) * P], ident[:])
                nc.scalar.activation(kvT_sb[:, ct, :], ptr[:],
                                     mybir.ActivationFunctionType.Identity)
            # k projection -> kT [d, h, m]
            kT_sb = kvpool.tile([DH, H, KM], BF, tag="kT")
            for h in range(H):
                pk = kvps.tile([DH, KM], F32, tag="pk")
                for ct in range(CT):
                    nc.tensor.matmul(pk[:], kw_sb[:, ct, h * DH:(h + 1) * DH],
                                     kvT_sb[:, ct, :],
                                     start=(ct == 0), stop=(ct == CT - 1))
                nc.scalar.activation(kT_sb[:, h, :], pk[:],
                                     mybir.ActivationFunctionType.Identity,
                                     bias=kb_sb[:, h:h + 1])
            # v projection -> v [m, c]
            pv1 = kvps.tile([KM, 512], F32, tag="pv1")
            pv2 = kvps.tile([KM, 256], F32, tag="pv2")
            for ct in range(CT):
                nc.tensor.matmul(pv1[:], kvT_sb[:, ct, :], vw_sb[:, ct, 0:512],
                                 start=(ct == 0), stop=(ct == CT - 1))
                nc.tensor.matmul(pv2[:], kvT_sb[:, ct, :], vw_sb[:, ct, 512:768],
                                 start=(ct == 0), stop=(ct == CT - 1))
            v_sb = kvpool.tile([KM, C], BF, tag="v")
            nc.vector.tensor_add(v_sb[:, 0:512], pv1[:], vb_b[:, 0:512])
            nc.vector.tensor_add(v_sb[:, 512:768], pv2[:], vb_b[:, 512:768])

            # block-diagonal v tiles: vblk[t] rows 0:64 = v of head hA in its
            # column range, rows 64:128 = v of head hB, zeros elsewhere
            vblk = kvpool.tile([P, CT, P], BF, tag="vblk")
            nc.vector.memset(vblk[:], 0.0)
            for t in range(CT):
                hA, wA = _tile_heads(t)
                c0 = P * t
                nc.vector.tensor_copy(vblk[0:KM, t, 0:wA], v_sb[:, c0:c0 + wA])
                nc.vector.tensor_copy(vblk[KM:P, t, wA:P],
                                      v_sb[:, c0 + wA:c0 + P])

            # W_pr^T[c, (mA|mB)] = sum_d qw[h]^T k[h]^T for the head pair
            W_sb = kvpool.tile([P, CT, 4, P], BF, tag="W")
            for pr in range(4):
                h0, h1 = 2 * pr, 2 * pr + 1
                for ct in range(CT):
                    wps = kvps.tile([P, P], F32, tag="wps")
                    nc.tensor.matmul(wps[:, 0:KM],
                                     qwh_sb[:, h0, ct * P:(ct + 1) * P],
                                     kT_sb[:, h0, :], start=True, stop=True)
                    nc.tensor.matmul(wps[:, KM:P],
                                     qwh_sb[:, h1, ct * P:(ct + 1) * P],
                                     kT_sb[:, h1, :], start=True, stop=True)
                    nc.scalar.activation(W_sb[:, ct, pr, :], wps[:],
                                         mybir.ActivationFunctionType.Identity)
            # per-pair exp bias: attnb + SCALE * (q_b . k_h[m])
            qkps = kvps.tile([P, 4], F32, tag="qkps")
            for pr in range(4):
                h0, h1 = 2 * pr, 2 * pr + 1
                nc.tensor.matmul(qkps[0:KM, pr:pr + 1], kT_sb[:, h0, :],
                                 qb_sb[:, h0:h0 + 1], start=True, stop=True)
                nc.tensor.matmul(qkps[KM:P, pr:pr + 1], kT_sb[:, h1, :],
                                 qb_sb[:, h1:h1 + 1], start=True, stop=True,
                                 tile_position=(0, KM))
            battn = kvpool.tile([P, 4], F32, tag="battn")
            for pr in range(4):
                nc.scalar.activation(battn[:, pr:pr + 1], qkps[:, pr:pr + 1],
                                     mybir.ActivationFunctionType.Identity,
                                     bias=attnb[:], scale=SCALE)
            kvps_ctx.__exit__(None, None, None)

            # ====== attention (pair-stacked) + output proj, per chunk ======
            apool = stack.enter_context(tc.tile_pool(name="attn", bufs=2))
            rpool = stack.enter_context(tc.tile_pool(name="rp", bufs=2))
            xpool = stack.enter_context(tc.tile_pool(name="x", bufs=2))
            opool = stack.enter_context(tc.tile_pool(name="ob", bufs=2))
            pss = stack.enter_context(tc.tile_pool(name="pss", bufs=2, space="PSUM"))
            psd = stack.enter_context(tc.tile_pool(name="psd", bufs=1, space="PSUM"))
            psx = stack.enter_context(tc.tile_pool(name="psx", bufs=2, space="PSUM"))
            pso1 = stack.enter_context(tc.tile_pool(name="pso1", bufs=2, space="PSUM"))
            pso2 = stack.enter_context(tc.tile_pool(name="pso2", bufs=1, space="PSUM"))

            for ch in range(NCH):
                n0 = ch * NCHUNK
                expS = []
                rec = []
                for pr in range(4):
                    ps_s = pss.tile([P, NCHUNK], F32, tag="s")
                    for ct in range(CT):
                        nc.tensor.matmul(ps_s[:], W_sb[:, ct, pr, :],
                                         qx_sb[:, ct, n0:n0 + NCHUNK],
                                         start=(ct == 0), stop=(ct == CT - 1))
                    e = apool.tile([P, NCHUNK], BF, tag=f"e{pr}",
                                   name=f"e{ch}_{pr}")
                    nc.scalar.activation(e[:], ps_s[:],
                                         mybir.ActivationFunctionType.Exp,
                                         bias=battn[:, pr:pr + 1], scale=SCALE)
                    expS.append(e)
                    ps_d = psd.tile([P, NCHUNK], F32, tag="d")
                    nc.tensor.matmul(ps_d[:], ones_blk[:], e[:],
                                     start=True, stop=True)
                    r = apool.tile([P, NCHUNK], F32, tag=f"r{pr}",
                                   name=f"r{ch}_{pr}")
                    nc.vector.reciprocal_approx_fast(r[:], ps_d[:])
                    rec.append(r)

                # R[0:64, t] = normP of head hA(t); R[64:128, t] = head hB(t)
                R = rpool.tile([P, CT, NCHUNK], BF, tag="R", name=f"R{ch}")
                for t in range(CT):
                    hA, _ = _tile_heads(t)
                    for dst0, hh in ((0, hA), (KM, hA + 1)):
                        pr, rb = hh // 2, KM * (hh % 2)
                        nc.vector.tensor_mul(
                            R[dst0:dst0 + KM, t, :],
                            expS[pr][rb:rb + KM, :], rec[pr][rb:rb + KM, :])

                x_sb = xpool.tile([P, CT, NCHUNK], BF, tag="x")
                for t in range(CT):
                    px = psx.tile([P, NCHUNK], F32, tag="px")
                    nc.tensor.matmul(px[:], vblk[:, t, :], R[:, t, :],
                                     start=True, stop=True)
                    nc.scalar.activation(x_sb[:, t, :], px[:],
                                         mybir.ActivationFunctionType.Identity)

                for nsub in range(4):
                    po1 = pso1.tile([P, 512], F32, tag="po1")
                    po2 = pso2.tile([P, 256], F32, tag="po2")
                    for g in range(CT):
                        lx = x_sb[:, g, nsub * P:(nsub + 1) * P]
                        nc.tensor.matmul(po1[:], lx, pw_sb[:, g, 0:512],
                                         start=(g == 0), stop=(g == CT - 1))
                        nc.tensor.matmul(po2[:], lx, pw_sb[:, g, 512:768],
                                         start=(g == 0), stop=(g == CT - 1))
                    ob = opool.tile([P, C], F32, tag="ob")
                    nc.vector.tensor_add(ob[:, 0:512], po1[:],
                                         pb_b[:, 0:512])
                    nc.vector.tensor_add(ob[:, 512:768], po2[:],
                                         pb_b[:, 512:768])
                    row = n0 + nsub * P
                    nc.sync.dma_start(out[row:row + P, :], ob[:])

    nc.compile()
    return nc


def _prep_inputs(qx, kvx, kv_bias, q_w, q_b, k_w, k_b, v_w, v_b,
                 proj_w, proj_b, conv_w, conv_b, ln_w, ln_b):
    """Shard + lay out the full inputs for the 8 cores."""
    f32 = np.float32
    kwT = np.ascontiguousarray(k_w.T).astype(BF_NP)
    vwT = np.ascontiguousarray(v_w.T).astype(BF_NP)
    pwT = np.ascontiguousarray(proj_w.T).astype(BF_NP)
    qwh = np.ascontiguousarray(
        q_w.reshape(H, DH, C).transpose(1, 0, 2)).astype(BF_NP)
    qb2 = np.ascontiguousarray(q_b.reshape(H, DH).T).astype(BF_NP)
    kb2 = np.ascontiguousarray(k_b.reshape(H, DH).T).astype(f32)

    # kvx pixel (y, x): y = 8*oy + dy, x = 8*ox + dx; core dy gets layout
    # [c, half(oy//4), dx, b, i2(oy%4), jj(ox)]
    kv6 = kvx.reshape(B, 2, 4, 8, 8, 8, C)  # [b, half, i2, dy, jj, dx, c]
    in_maps = []
    for core in range(8):
        kvg = np.ascontiguousarray(
            kv6[:, :, :, core].transpose(5, 1, 4, 0, 2, 3).reshape(C, 4096)
        ).astype(BF_NP)
        cwT = np.ascontiguousarray(
            conv_w[:, :, core, :].transpose(2, 1, 0)).astype(BF_NP)
        in_maps.append({
            "qxT": np.ascontiguousarray(qx[core].T).astype(BF_NP),
            "kvg": kvg,
            "cwT": cwT,
            "kvb": np.ascontiguousarray(kv_bias[core, 0]).astype(f32),
            "qwh": qwh, "kwT": kwT, "vwT": vwT, "pwT": pwT,
            "qb2": qb2, "kb2": kb2,
            "vb": v_b.astype(f32), "cb": conv_b.astype(f32),
            "pb": proj_b.astype(f32),
            "lnw": ln_w.astype(f32), "lnb": ln_b.astype(f32),
        })
    return in_maps


def _run(inputs: dict, trace: bool = False):
    if "nc" not in _CACHE:
        _CACHE["nc"] = _build_program()
    nc = _CACHE["nc"]
    in_maps = _prep_inputs(
        qx=np.asarray(inputs["qx"]), kvx=np.asarray(inputs["kvx"]),
        kv_bias=np.asarray(inputs["kv_bias"]),
        q_w=np.asarray(inputs["q_w"]), q_b=np.asarray(inputs["q_b"]),
        k_w=np.asarray(inputs["k_w"]), k_b=np.asarray(inputs["k_b"]),
        v_w=np.asarray(inputs["v_w"]), v_b=np.asarray(inputs["v_b"]),
        proj_w=np.asarray(inputs["proj_w"]), proj_b=np.asarray(inputs["proj_b"]),
        conv_w=np.asarray(inputs["conv_w"]), conv_b=np.asarray(inputs["conv_b"]),
        ln_w=np.asarray(inputs["ln_w"]), ln_b=np.asarray(inputs["ln_b"]))
    res = run_bass_kernel_spmd(nc, in_maps, core_ids=list(range(8)), trace=trace)
    full = np.stack([res.results[c]["out"] for c in range(8)], axis=0)
    return full.astype(np.float32), res


def kernel(**inputs) -> np.ndarray:
    full, _ = _run(inputs, trace=False)
    return full


# revision 35
# speedup vs baseline: 1.0424x; 1.0424x over previous
"""Trainium2 Bass kernel for nn_CrossSRA (spatial-reduction cross-attention).

Sharding (8 NeuronCores):
  - Batch-parallel for the attention path: core b owns batch b.
  - The spatial-reduction conv is split by kernel-row dy across the 8 cores;
    partials are combined with two in-kernel ReduceScatters (token-split so
    RS#1 overlaps the second half of the conv).

Key structure:
  - q-projection is folded into the score matmuls: W_pr = [qw_h^T k_h^T]
    precomputed per head-pair, scores = W^T @ qxT (pair-stacked PSUM).
  - Softmax denominators via one block-diagonal ones matmul per pair.
  - attn@v via 6 block-diagonal v-tile matmuls per chunk.
All matmuls bf16 with fp32 PSUM; layernorm/softmax statistics stay fp32.
"""

import numpy as np
import ml_dtypes

import concourse.bass as bass
import concourse.tile as tile
from concourse import bacc, mybir
from concourse.bass_utils import run_bass_kernel_spmd
from concourse.masks import make_identity

# problem shape (hardcoded per spec)
B = 8
N = 4096
C = 768
H = 8
DH = C // H            # 96
IMG = 64               # h = w = 64
SR = 8
KM = 64                # kv tokens after spatial reduction (8x8)
EPS = 1e-5
SCALE = DH ** -0.5

P = 128
CT = C // P            # 6 channel tiles
NCHUNK = 512
NCH = N // NCHUNK      # 8 column chunks
HKM = KM // 2          # 32 tokens per conv half

BF = mybir.dt.bfloat16
F32 = mybir.dt.float32
BF_NP = ml_dtypes.bfloat16

_CACHE: dict = {}

# output channel tile t is covered by heads (hA, hA+1) split at width WA
def _tile_heads(t):
    c0 = P * t
    hA = c0 // DH
    wA = DH * (hA + 1) - c0
    return hA, wA


def _build_program():
    nc = bacc.Bacc("TRN2", target_bir_lowering=False, debug=False, num_devices=8)

    d_in = {}
    def din(name, shape, dt):
        d_in[name] = nc.dram_tensor(name, shape, dt, kind="ExternalInput").ap()
        return d_in[name]

    qxT = din("qxT", [C, N], BF)          # this batch's qx, transposed
    # all batches' kvx tokens with dy=core, layout [c, half, dx, b, i2, jj]
    kvg = din("kvg", [C, 2 * SR * B * HKM], BF)  # [768, 4096]
    cwT = din("cwT", [SR, C, C], BF)      # conv_w[o, c, dy=core, dx] -> [dx, c, o]
    kvb = din("kvb", [IMG, IMG], F32)     # this batch's kv_bias image
    qwh = din("qwh", [DH, H, C], BF)      # q_w rows grouped per head: [d, h, c]
    qwT = din("qwT", [C, C], BF)          # q_w.T (for the qT-projected heads)
    kwT = din("kwT", [C, C], BF)
    vwT = din("vwT", [C, C], BF)
    pwT = din("pwT", [C, C], BF)          # proj_w.T
    qb2 = din("qb2", [DH, H], BF)
    kb2 = din("kb2", [DH, H], F32)
    vb = din("vb", [C], F32)
    cb = din("cb", [C], F32)
    pb = din("pb", [C], F32)
    lnw = din("lnw", [C], F32)
    lnb = din("lnb", [C], F32)

    out = nc.dram_tensor("out", [N, C], F32, kind="ExternalOutput").ap()

    def bcast(vec_ap, parts):
        return bass.AP(tensor=vec_ap.tensor, offset=0, ap=[[0, parts], [1, C]])

    with tile.TileContext(nc) as tc:
        import contextlib
        stack = contextlib.ExitStack()
        with stack:
            consts = stack.enter_context(tc.tile_pool(name="consts", bufs=1))
            wpool = stack.enter_context(tc.tile_pool(name="weights", bufs=1))
            dram = stack.enter_context(tc.tile_pool(name="dram", bufs=1, space="DRAM"))

            # ---- constants (scalar DMA queue; sync queue is conv-critical) ----
            ident = consts.tile([KM, KM], F32, tag="ident")
            make_identity(nc, ident[:])
            eps_t = consts.tile([KM, 1], F32, tag="eps")
            nc.vector.memset(eps_t[:], EPS)
            # block-diagonal ones: col c sums rows of its own half
            ones_blk = consts.tile([P, P], BF, tag="onesblk")
            nc.vector.memset(ones_blk[:], 0.0)
            nc.vector.memset(ones_blk[0:KM, 0:KM], 1.0)
            nc.vector.memset(ones_blk[KM:P, KM:P], 1.0)

            vb_b = consts.tile([KM, C], F32, tag="vb")
            nc.scalar.dma_start(vb_b[:], bcast(vb, KM))
            cb_b = consts.tile([KM, C], F32, tag="cb")
            nc.scalar.dma_start(cb_b[:], bcast(cb, KM))
            lnw_b = consts.tile([KM, C], F32, tag="lnw")
            nc.scalar.dma_start(lnw_b[:], bcast(lnw, KM))
            lnb_b = consts.tile([KM, C], F32, tag="lnb")
            nc.scalar.dma_start(lnb_b[:], bcast(lnb, KM))
            pb_b = consts.tile([P, C], F32, tag="pb")
            nc.scalar.dma_start(pb_b[:], bcast(pb, P))
            qb_sb = consts.tile([DH, H], BF, tag="qb")
            nc.scalar.dma_start(qb_sb[:], qb2[:])
            kb_sb = consts.tile([DH, H], F32, tag="kb")
            nc.scalar.dma_start(kb_sb[:], kb2[:])

            # attention bias: 4-point average of the bilinear resize (64->8)
            g4 = consts.tile([8, 8, 2, 2], F32, tag="g4")
            for dy in range(2):
                src = bass.AP(tensor=kvb.tensor, offset=(3 + dy) * IMG + 3,
                              ap=[[8 * IMG, 8], [8, 8], [1, 2]])
                nc.scalar.dma_start(g4[:, :, dy, :], src)
            s4 = consts.tile([8, 8], F32, tag="s4")
            nc.vector.reduce_sum(s4[:], g4[:], axis=mybir.AxisListType.XY)
            s4q = consts.tile([8, 8], F32, tag="s4q")
            nc.scalar.mul(s4q[:], s4[:], 0.25)
            attnb = consts.tile([P, 1], F32, tag="attnb")
            nc.scalar.dma_start(attnb[0:KM, :], s4q[:])   # [8p,8f] -> [64p,1f]
            nc.scalar.dma_start(attnb[KM:P, :], s4q[:])   # duplicate for pair stack

            # collective bounce buffers (bf16 partial sums), rows = (b, m)
            partials = dram.tile([B * KM, C], BF, tag="partials",
                                 name="partials")
            kvred = dram.tile([KM, C], BF, tag="kvred", name="kvred")

            # qx resident for the whole attention phase; its DMAs are queued
            # on sync AFTER the conv stream, weights after that
            qxpool = stack.enter_context(tc.tile_pool(name="qx", bufs=1))
            qx_sb = qxpool.tile([P, CT, N], BF, tag="qxall")

            # ================= conv phase (k-split over dy) =================
            with tc.tile_pool(name="convp", bufs=8) as cvpool, \
                 tc.tile_pool(name="cwres", bufs=1) as cwpool, \
                 tc.tile_pool(name="cvpsum", bufs=2, space="PSUM") as cvps, \
                 tc.tile_pool(name="cvout", bufs=2) as cvo:
                # sync-queue FIFO in consumption order: (cw_dx, kvx_h0_dx)
                # interleaved, then kvx_h1_dx, then qx, then weights
                cw_t = []
                kvx_t = {}
                for dx in range(SR):
                    t = cwpool.tile([P, CT, C], BF, tag=f"cw{dx}", name=f"cw{dx}")
                    kt = cvpool.tile([P, CT, 256], BF, tag="kvx",
                                     name=f"kvx0_{dx}")
                    cw_view = bass.AP(tensor=cwT.tensor, offset=dx * C * C,
                                      ap=[[C, P], [P * C, CT], [1, C]])
                    kv_view = bass.AP(tensor=kvg.tensor, offset=dx * 256,
                                      ap=[[4096, P], [P * 4096, CT], [1, 256]])
                    if dx == 0:
                        # per-ct interleaved so the first matmuls unblock early
                        for ct in range(CT):
                            cw_ct = bass.AP(tensor=cwT.tensor,
                                            offset=dx * C * C + ct * P * C,
                                            ap=[[C, P], [1, C]])
                            nc.sync.dma_start(t[:, ct, :], cw_ct)
                            kv_ct = bass.AP(tensor=kvg.tensor,
                                            offset=dx * 256 + ct * P * 4096,
                                            ap=[[4096, P], [1, 256]])
                            nc.sync.dma_start(kt[:, ct, :], kv_ct)
                    else:
                        nc.sync.dma_start(t[:], cw_view)
                        nc.sync.dma_start(kt[:], kv_view)
                    cw_t.append(t)
                    kvx_t[(0, dx)] = kt
                for dx in range(SR):
                    kt = cvpool.tile([P, CT, 256], BF, tag="kvx",
                                     name=f"kvx1_{dx}")
                    view = bass.AP(tensor=kvg.tensor, offset=2048 + dx * 256,
                                   ap=[[4096, P], [P * 4096, CT], [1, 256]])
                    nc.sync.dma_start(kt[:], view)
                    kvx_t[(1, dx)] = kt
                for half in range(2):
                    cps = [cvps.tile([P, C], F32, tag=f"cv{q}", name=f"cv{half}{q}")
                           for q in range(2)]
                    for dx in range(SR):
                        for q in range(2):
                            for ct in range(CT):
                                lhsT = kvx_t[(half, dx)][:, ct,
                                                         q * P:(q + 1) * P]
                                for o0, osz in ((0, 512), (512, 256)):
                                    nc.tensor.matmul(
                                        cps[q][:, o0:o0 + osz], lhsT,
                                        cw_t[dx][:, ct, o0:o0 + osz],
                                        start=(dx == 0 and ct == 0),
                                        stop=(dx == SR - 1 and ct == CT - 1))
                    for q in range(2):
                        pt = cvo.tile([P, C], BF, tag="cvo")
                        nc.vector.tensor_copy(pt[:], cps[q][:])
                        # psum rows (b-4q, i2, jj) -> partials rows b*64+32h+..
                        dst = bass.AP(
                            tensor=partials[:].tensor,
                            offset=(q * 4 * KM + HKM * half) * C,
                            ap=[[KM * C, 4], [C, HKM], [1, C]])
                        nc.gpsimd.dma_start(dst, pt[:])
                nc.gpsimd.collective_compute(
                    "ReduceScatter", mybir.AluOpType.add,
                    replica_groups=[list(range(8))],
                    ins=[partials.opt()], outs=[kvred.opt()])
                # dedent note: single RS after both halves (see loop below)

            # remaining weights (sync FIFO: after conv stream + qx)
            def load_wT(src_ap, tag):
                t = wpool.tile([P, CT, C], BF, tag=tag)
                view = bass.AP(tensor=src_ap.tensor, offset=0,
                               ap=[[C, P], [P * C, CT], [1, C]])
                nc.sync.dma_start(t[:], view)
                return t

            # q_w.T columns for heads 0-3 only (the qT-projected half)
            qwT_sb = wpool.tile([P, CT, 4 * DH], BF, tag="qwT")
            qwT_view = bass.AP(tensor=qwT.tensor, offset=0,
                               ap=[[C, P], [P * C, CT], [1, 4 * DH]])
            nc.sync.dma_start(qwT_sb[:], qwT_view)
            qwh_sb = wpool.tile([DH, H, C], BF, tag="qwh")
            nc.sync.dma_start(qwh_sb[:], qwh[:])
            kw_sb = load_wT(kwT, "kw")
            vw_sb = load_wT(vwT, "vw")
            pw_sb = load_wT(pwT, "pw")
            for ct in range(CT):
                view = bass.AP(tensor=qxT.tensor, offset=ct * P * N,
                               ap=[[N, P], [1, N]])
                nc.sync.dma_start(qx_sb[:, ct, :], view)

            # f32 copy of q_b for the qproj activation bias
            qbf = consts.tile([DH, H], F32, tag="qbf")
            nc.vector.tensor_copy(qbf[:], qb_sb[:])

            # ======= q projection for heads 0-3 (fills the RS wait on PE) ====
            # Emitted before the RS-gated epilogue so the in-order PE queue
            # can run it while the collective is in flight.
            qtpool = stack.enter_context(tc.tile_pool(name="qT96", bufs=1))
            qT96 = qtpool.tile([DH, 4, N], BF, tag="qT96")
            GRP = 4
            with tc.tile_pool(name="qpsum", bufs=2, space="PSUM") as qps:
                for h in range(4):
                    for g in range(NCH // GRP):
                        pq = qps.tile([DH, GRP, NCHUNK], F32, tag="pq")
                        for ct in range(CT):
                            for cc in range(GRP):
                                n0 = (g * GRP + cc) * NCHUNK
                                nc.tensor.matmul(
                                    pq[:, cc, :],
                                    qwT_sb[:, ct, h * DH:(h + 1) * DH],
                                    qx_sb[:, ct, n0:n0 + NCHUNK],
                                    start=(ct == 0), stop=(ct == CT - 1))
                        nc.scalar.activation(
                            qT96[:, h, g * GRP * NCHUNK:(g + 1) * GRP * NCHUNK],
                            pq[:].rearrange("p g n -> p (g n)"),
                            mybir.ActivationFunctionType.Identity,
                            bias=qbf[:, h:h + 1])

            # ================= kv epilogue =================
            kvpool = stack.enter_context(tc.tile_pool(name="kv", bufs=1))
            kvps_ctx = tc.tile_pool(name="kvpsum", bufs=1, space="PSUM")
            kvps = kvps_ctx.__enter__()
            kvbf = kvpool.tile([KM, C], BF, tag="kvbf")
            nc.gpsimd.dma_start(kvbf[:], kvred[:])
            kv_sb = kvpool.tile([KM, C], F32, tag="kv")
            nc.vector.tensor_add(kv_sb[:], kvbf[:], cb_b[:])
            # layernorm over channels
            BD = nc.vector.BN_STATS_DIM
            stats = kvpool.tile([KM, 3, BD], F32, tag="stats")
            kv_g = kv_sb[:].rearrange("p (g d) -> p g d", g=3)
            for g in range(3):
                nc.vector.bn_stats(stats[:, g, :], kv_g[:, g, :])
            mv = kvpool.tile([KM, nc.vector.BN_AGGR_DIM], F32, tag="mv")
            nc.vector.bn_aggr(mv[:], stats[:])
            nc.scalar.activation(mv[:, 1:2], mv[:, 1:2],
                                 mybir.ActivationFunctionType.Sqrt, bias=eps_t[:])
            nc.vector.reciprocal(mv[:, 1:2], mv[:, 1:2])
            nc.vector.tensor_scalar(kv_sb[:], kv_sb[:],
                                    scalar1=mv[:, 0:1], scalar2=mv[:, 1:2],
                                    op0=mybir.AluOpType.subtract,
                                    op1=mybir.AluOpType.mult)
            nc.vector.tensor_mul(kv_sb[:], kv_sb[:], lnw_b[:])
            nc.vector.tensor_add(kv_sb[:], kv_sb[:], lnb_b[:])

            # transpose kv -> kvT [c, m]
            kvT_sb = kvpool.tile([P, CT, KM], BF, tag="kvT")
            for ct in range(CT):
                ptr = kvps.tile([P, KM], F32, tag="ptr")
                nc.tensor.transpose(ptr[:], kv_sb[:, ct * P:(ct + 1) * P], ident[:])
                nc.scalar.activation(kvT_sb[:, ct, :], ptr[:],
                                     mybir.ActivationFunctionType.Identity)
            # k projection -> kT [d, h, m]
            kT_sb = kvpool.tile([DH, H, KM], BF, tag="kT")
            for h in range(H):
                pk = kvps.tile([DH, KM], F32, tag="pk")
                for ct in range(CT):
                    nc.tensor.matmul(pk[:], kw_sb[:, ct, h * DH:(h + 1) * DH],
                                     kvT_sb[:, ct, :],
                                     start=(ct == 0), stop=(ct == CT - 1))
                nc.scalar.activation(kT_sb[:, h, :], pk[:],
                                     mybir.ActivationFunctionType.Identity,
                                     bias=kb_sb[:, h:h + 1])
            # v projection -> v [m, c]
            pv1 = kvps.tile([KM, 512], F32, tag="pv1")
            pv2 = kvps.tile([KM, 256], F32, tag="pv2")
            for ct in range(CT):
                nc.tensor.matmul(pv1[:], kvT_sb[:, ct, :], vw_sb[:, ct, 0:512],
                                 start=(ct == 0), stop=(ct == CT - 1))
                nc.tensor.matmul(pv2[:], kvT_sb[:, ct, :], vw_sb[:, ct, 512:768],
                                 start=(ct == 0), stop=(ct == CT - 1))
            v_sb = kvpool.tile([KM, C], BF, tag="v")
            nc.vector.tensor_add(v_sb[:, 0:512], pv1[:], vb_b[:, 0:512])
            nc.vector.tensor_add(v_sb[:, 512:768], pv2[:], vb_b[:, 512:768])

            # block-diagonal v tiles: vblk[t] rows 0:64 = v of head hA in its
            # column range, rows 64:128 = v of head hB, zeros elsewhere
            vblk = kvpool.tile([P, CT, P], BF, tag="vblk")
            nc.vector.memset(vblk[:], 0.0)
            for t in range(CT):
                hA, wA = _tile_heads(t)
                c0 = P * t
                nc.vector.tensor_copy(vblk[0:KM, t, 0:wA], v_sb[:, c0:c0 + wA])
                nc.vector.tensor_copy(vblk[KM:P, t, wA:P],
                                      v_sb[:, c0 + wA:c0 + P])

            # W_pr^T[c, (mA|mB)] = sum_d qw[h]^T k[h]^T for head pairs 2,3
            W_sb = kvpool.tile([P, CT, 2, P], BF, tag="W")
            for pw_i, pr in enumerate((2, 3)):
                h0, h1 = 2 * pr, 2 * pr + 1
                for ct in range(CT):
                    wps = kvps.tile([P, P], F32, tag="wps")
                    nc.tensor.matmul(wps[:, 0:KM],
                                     qwh_sb[:, h0, ct * P:(ct + 1) * P],
                                     kT_sb[:, h0, :], start=True, stop=True)
                    nc.tensor.matmul(wps[:, KM:P],
                                     qwh_sb[:, h1, ct * P:(ct + 1) * P],
                                     kT_sb[:, h1, :], start=True, stop=True)
                    nc.scalar.activation(W_sb[:, ct, pw_i, :], wps[:],
                                         mybir.ActivationFunctionType.Identity)
            # pair exp bias for W-form pairs: attnb + SCALE * (q_b . k_h[m])
            qkps = kvps.tile([P, 2], F32, tag="qkps")
            for pw_i, pr in enumerate((2, 3)):
                h0, h1 = 2 * pr, 2 * pr + 1
                nc.tensor.matmul(qkps[0:KM, pw_i:pw_i + 1], kT_sb[:, h0, :],
                                 qb_sb[:, h0:h0 + 1], start=True, stop=True)
                nc.tensor.matmul(qkps[KM:P, pw_i:pw_i + 1], kT_sb[:, h1, :],
                                 qb_sb[:, h1:h1 + 1], start=True, stop=True,
                                 tile_position=(0, KM))
            battn = kvpool.tile([P, 2], F32, tag="battn")
            for pw_i in range(2):
                nc.scalar.activation(battn[:, pw_i:pw_i + 1],
                                     qkps[:, pw_i:pw_i + 1],
                                     mybir.ActivationFunctionType.Identity,
                                     bias=attnb[:], scale=SCALE)
            kvps_ctx.__exit__(None, None, None)

            # ====== attention (pair-stacked) + output proj, per chunk ======
            apool = stack.enter_context(tc.tile_pool(name="attn", bufs=2))
            rpool = stack.enter_context(tc.tile_pool(name="rp", bufs=2))
            xpool = stack.enter_context(tc.tile_pool(name="x", bufs=2))
            opool = stack.enter_context(tc.tile_pool(name="ob", bufs=2))
            pss = stack.enter_context(tc.tile_pool(name="pss", bufs=2, space="PSUM"))
            psd = stack.enter_context(tc.tile_pool(name="psd", bufs=1, space="PSUM"))
            psx = stack.enter_context(tc.tile_pool(name="psx", bufs=2, space="PSUM"))
            pso1 = stack.enter_context(tc.tile_pool(name="pso1", bufs=2, space="PSUM"))
            pso2 = stack.enter_context(tc.tile_pool(name="pso2", bufs=1, space="PSUM"))

            for ch in range(NCH):
                n0 = ch * NCHUNK
                expS = []
                rec = []
                for pr in range(4):
                    ps_s = pss.tile([P, NCHUNK], F32, tag="s")
                    if pr < 2:
                        # qT-projected heads: classic pair-stacked scores
                        h0, h1 = 2 * pr, 2 * pr + 1
                        nc.tensor.matmul(ps_s[0:KM, :], kT_sb[:, h0, :],
                                         qT96[:, h0, n0:n0 + NCHUNK],
                                         start=True, stop=True)
                        nc.tensor.matmul(ps_s[KM:P, :], kT_sb[:, h1, :],
                                         qT96[:, h1, n0:n0 + NCHUNK],
                                         start=True, stop=True,
                                         tile_position=(0, KM))
                        bias_ap = attnb[:]
                    else:
                        for ct in range(CT):
                            nc.tensor.matmul(ps_s[:], W_sb[:, ct, pr - 2, :],
                                             qx_sb[:, ct, n0:n0 + NCHUNK],
                                             start=(ct == 0),
                                             stop=(ct == CT - 1))
                        bias_ap = battn[:, pr - 2:pr - 1]
                    e = apool.tile([P, NCHUNK], BF, tag=f"e{pr}",
                                   name=f"e{ch}_{pr}")
                    nc.scalar.activation(e[:], ps_s[:],
                                         mybir.ActivationFunctionType.Exp,
                                         bias=bias_ap, scale=SCALE)
                    expS.append(e)
                    ps_d = psd.tile([P, NCHUNK], F32, tag="d")
                    nc.tensor.matmul(ps_d[:], ones_blk[:], e[:],
                                     start=True, stop=True)
                    r = apool.tile([P, NCHUNK], F32, tag=f"r{pr}",
                                   name=f"r{ch}_{pr}")
                    nc.vector.reciprocal_approx_fast(r[:], ps_d[:])
                    rec.append(r)

                # R[0:64, t] = normP of head hA(t); R[64:128, t] = head hB(t)
                R = rpool.tile([P, CT, NCHUNK], BF, tag="R", name=f"R{ch}")
                for t in range(CT):
                    hA, _ = _tile_heads(t)
                    for dst0, hh in ((0, hA), (KM, hA + 1)):
                        pr, rb = hh // 2, KM * (hh % 2)
                        nc.vector.tensor_mul(
                            R[dst0:dst0 + KM, t, :],
                            expS[pr][rb:rb + KM, :], rec[pr][rb:rb + KM, :])

                x_sb = xpool.tile([P, CT, NCHUNK], BF, tag="x")
                for t in range(CT):
                    px = psx.tile([P, NCHUNK], F32, tag="px")
                    nc.tensor.matmul(px[:], vblk[:, t, :], R[:, t, :],
                                     start=True, stop=True)
                    nc.scalar.activation(x_sb[:, t, :], px[:],
                                         mybir.ActivationFunctionType.Identity)

                for nsub in range(4):
                    po1 = pso1.tile([P, 512], F32, tag="po1")
                    po2 = pso2.tile([P, 256], F32, tag="po2")
                    for g in range(CT):
                        lx = x_sb[:, g, nsub * P:(nsub + 1) * P]
                        nc.tensor.matmul(po1[:], lx, pw_sb[:, g, 0:512],
                                         start=(g == 0), stop=(g == CT - 1))
                        nc.tensor.matmul(po2[:], lx, pw_sb[:, g, 512:768],
                                         start=(g == 0), stop=(g == CT - 1))
                    ob = opool.tile([P, C], F32, tag="ob")
                    nc.vector.tensor_add(ob[:, 0:512], po1[:],
                                         pb_b[:, 0:512])
                    nc.vector.tensor_add(ob[:, 512:768], po2[:],
                                         pb_b[:, 512:768])
                    row = n0 + nsub * P
                    nc.sync.dma_start(out[row:row + P, :], ob[:])

    nc.compile()
    return nc


def _prep_inputs(qx, kvx, kv_bias, q_w, q_b, k_w, k_b, v_w, v_b,
                 proj_w, proj_b, conv_w, conv_b, ln_w, ln_b):
    """Shard + lay out the full inputs for the 8 cores."""
    f32 = np.float32
    kwT = np.ascontiguousarray(k_w.T).astype(BF_NP)
    vwT = np.ascontiguousarray(v_w.T).astype(BF_NP)
    pwT = np.ascontiguousarray(proj_w.T).astype(BF_NP)
    qwh = np.ascontiguousarray(
        q_w.reshape(H, DH, C).transpose(1, 0, 2)).astype(BF_NP)
    qwT = np.ascontiguousarray(q_w.T).astype(BF_NP)
    qb2 = np.ascontiguousarray(q_b.reshape(H, DH).T).astype(BF_NP)
    kb2 = np.ascontiguousarray(k_b.reshape(H, DH).T).astype(f32)

    # kvx pixel (y, x): y = 8*oy + dy, x = 8*ox + dx; core dy gets layout
    # [c, half(oy//4), dx, b, i2(oy%4), jj(ox)]
    kv6 = kvx.reshape(B, 2, 4, 8, 8, 8, C)  # [b, half, i2, dy, jj, dx, c]
    in_maps = []
    for core in range(8):
        kvg = np.ascontiguousarray(
            kv6[:, :, :, core].transpose(5, 1, 4, 0, 2, 3).reshape(C, 4096)
        ).astype(BF_NP)
        cwT = np.ascontiguousarray(
            conv_w[:, :, core, :].transpose(2, 1, 0)).astype(BF_NP)
        in_maps.append({
            "qxT": np.ascontiguousarray(qx[core].T).astype(BF_NP),
            "kvg": kvg,
            "cwT": cwT,
            "kvb": np.ascontiguousarray(kv_bias[core, 0]).astype(f32),
            "qwh": qwh, "qwT": qwT, "kwT": kwT, "vwT": vwT, "pwT": pwT,
            "qb2": qb2, "kb2": kb2,
            "vb": v_b.astype(f32), "cb": conv_b.astype(f32),
            "pb": proj_b.astype(f32),
            "lnw": ln_w.astype(f32), "lnb": ln_b.astype(f32),
        })
    return in_maps


def _run(inputs: dict, trace: bool = False):
    if "nc" not in _CACHE:
        _CACHE["nc"] = _build_program()
    nc = _CACHE["nc"]
    in_maps = _prep_inputs(
        qx=np.asarray(inputs["qx"]), kvx=np.asarray(inputs["kvx"]),
        kv_bias=np.asarray(inputs["kv_bias"]),
        q_w=np.asarray(inputs["q_w"]), q_b=np.asarray(inputs["q_b"]),
        k_w=np.asarray(inputs["k_w"]), k_b=np.asarray(inputs["k_b"]),
        v_w=np.asarray(inputs["v_w"]), v_b=np.asarray(inputs["v_b"]),
        proj_w=np.asarray(inputs["proj_w"]), proj_b=np.asarray(inputs["proj_b"]),
        conv_w=np.asarray(inputs["conv_w"]), conv_b=np.asarray(inputs["conv_b"]),
        ln_w=np.asarray(inputs["ln_w"]), ln_b=np.asarray(inputs["ln_b"]))
    res = run_bass_kernel_spmd(nc, in_maps, core_ids=list(range(8)), trace=trace)
    full = np.stack([res.results[c]["out"] for c in range(8)], axis=0)
    return full.astype(np.float32), res


def kernel(**inputs) -> np.ndarray:
    full, _ = _run(inputs, trace=False)
    return full
